# revision 9
# baseline (speedup 1.0000x reference)
"""GCNConv (COO SpMM aggregation + dense GEMM) on 8 Trainium2 NeuronCores.

  msgs = edge_vals[:, None] * x[edge_col]          # [E, 64] gather+scale
  agg  = segment_sum(msgs, edge_row, N)            # [N, 64] scatter-add
  out  = agg @ weight                              # [N, 64] GEMM

Sharding: destination-node sharding (each core owns a contiguous row slab and
all edges targeting it) -> zero collectives.

The throughput limit is SWDGE descriptor generation for the per-edge row
gather (~7.75 ns/descriptor + ~540 ns/call on the Q7s), so the host-side prep
minimizes padded gather slots:
  - x is split into 4 unequal quarters [23040,23040,23040,31232] (int16
    gather indices), sized so each (block, quarter) edge-group mean sits
    well below a multiple of 128.
  - each core's 12544 rows are bin-packed into 98 blocks of 128 rows,
    balancing all 4 per-quarter degree sums, so the max (block, quarter)
    group stays within [512,512,512,768] slots (vs 640 for equal quarters
    without packing).  The row permutation is undone on the host at the end.

Per core:
  - gpsimd.dma_gather pulls 64-float source rows (256B each) from HBM
  - one DVE tensor_tensor per (supergroup, quarter) scales msgs by edge_vals
  - one DVE tensor_tensor per dest block builds the one-hot
    oh[e, r] = (dest[e] == r) via iota-compare with a broadcast dest AP
  - TensorE per 128-edge chunk: psum[128 rows, 64] += oh.T @ msgs
  - per block: PE-transpose agg -> aggT, then outT[64,128] = W.T @ aggT
  - one contiguous [64, rows] output DMA; host scatters rows back.
"""

import os
import sys

import numpy as np

if "/opt/trn_rl_repo" not in sys.path:
    sys.path.insert(0, "/opt/trn_rl_repo")

# ---------------------------------------------------------------- constants
N = 100000
E = 1600000
D = 64
CORES = 8
RPC = 12544          # rows per core (8*12544 = 100352 >= N)
BLOCKS = RPC // 128  # 98 dest blocks per core
Q = 4
QS = np.array([0, 30134, 54243, 78352, 100352], dtype=np.int64)  # quarter bounds
CAPQ = np.array([640, 512, 512, 512], dtype=np.int64)  # packing targets
G = 7                # dest blocks per gather super-group (98 = 14*7)
NGROUPS = BLOCKS // G

LAST_EXEC_TIME_NS = None
_CACHE = {}


def _pack_rows(deg):
    """Assign RPC rows (deg: [RPC, 4] per-quarter degrees) to BLOCKS blocks
    of 128, balancing all 4 quarter sums against the CAPQ targets.  Greedy
    rounds (one row per block per round) + peak-shaving swap repair.
    Returns perm_local[pos] = row, where pos = block*128 + slot."""
    order = np.argsort(-deg.sum(1), kind="stable")
    cur = np.zeros((BLOCKS, Q), np.float64)
    capf = CAPQ.astype(np.float64)
    blk_of = np.empty(RPC, np.int64)
    for rnd in range(128):
        batch = order[rnd * BLOCKS : (rnd + 1) * BLOCKS]
        bscore = (deg[batch] / capf).max(1)
        bo = batch[np.argsort(-bscore, kind="stable")]
        load = (cur / capf).max(1)
        blko = np.argsort(load, kind="stable")
        cur[blko] += deg[bo]
        blk_of[bo] = blko
    # repair: swap the heaviest row (in the hottest quarter) of the hottest
    # block with a light row of the coolest block
    loadi = np.zeros((BLOCKS, Q), np.int64)
    np.add.at(loadi, blk_of, deg)
    rows_in = [list(np.where(blk_of == b)[0]) for b in range(BLOCKS)]
    for _ in range(4000):
        nl = loadi / capf
        b, q = np.unravel_index(np.argmax(nl), nl.shape)
        b, q = int(b), int(q)
        if nl[b, q] <= 1.0:
            break
        cand = max(rows_in[b], key=lambda r: deg[r, q])
        tgt = int(np.argmin(nl[:, q] + (np.arange(BLOCKS) == b) * 10))
        cand2 = min(rows_in[tgt], key=lambda r: deg[r, q])
        loadi[b] += deg[cand2] - deg[cand]
        loadi[tgt] += deg[cand] - deg[cand2]
        rows_in[b].remove(cand)
        rows_in[b].append(cand2)
        rows_in[tgt].remove(cand2)
        rows_in[tgt].append(cand)
    perm_local = np.empty(RPC, np.int64)
    for b in range(BLOCKS):
        for j, r in enumerate(rows_in[b]):
            perm_local[b * 128 + j] = r
    return perm_local


# ---------------------------------------------------------------- host prep
def _prep(x, weight, edge_vals, edge_row, edge_col):
    e_row = np.asarray(edge_row, dtype=np.int64)
    e_col = np.asarray(edge_col, dtype=np.int64)
    ev = np.asarray(edge_vals, dtype=np.float32)
    x = np.asarray(x, dtype=np.float32)
    weight = np.asarray(weight, dtype=np.float32)
    ne = e_row.shape[0]
    NPAD = CORES * RPC

    qq = np.searchsorted(QS, e_col, side="right") - 1
    lidx = (e_col - QS[qq]).astype(np.int16)

    # per-row per-quarter degrees -> per-core packing permutation
    deg_flat = np.bincount(e_row * Q + qq, minlength=NPAD * Q).reshape(NPAD, Q)
    perm = np.empty((CORES, RPC), np.int64)      # perm[k, pos] = global row
    pos_of_row = np.empty(NPAD, np.int64)        # core-local position
    for k in range(CORES):
        pl = _pack_rows(deg_flat[k * RPC : (k + 1) * RPC])
        perm[k] = k * RPC + pl
        pos_of_row[perm[k]] = np.arange(RPC)

    core = e_row // RPC
    pos = pos_of_row[e_row]
    blk = pos // 128
    dest = (pos % 128).astype(np.float32)

    # group counts -> per-quarter chunk counts (global static)
    gkey = (core * BLOCKS + blk) * Q + qq
    counts = np.bincount(gkey, minlength=CORES * BLOCKS * Q)
    cmax = counts.reshape(CORES * BLOCKS, Q).max(axis=0)
    Cq = np.maximum(1, -(-cmax // 128))          # [Q] chunks per group
    SLq = Cq * 128
    SLOTSB = int(SLq.sum())                      # slots per block
    NCH = int(Cq.sum())                          # chunk-columns per block
    qslotoff = np.concatenate([[0], np.cumsum(SLq)[:-1]])

    order = np.argsort(gkey, kind="stable")
    NGK = CORES * BLOCKS * Q
    starts = np.zeros(NGK, np.int64)
    starts[1:] = np.cumsum(counts)[:-1]
    gsort = gkey[order]
    rank = np.arange(ne, dtype=np.int64) - starts[gsort]
    cb = gsort // Q
    qs = gsort % Q
    slot = cb * SLOTSB + qslotoff[qs] + rank

    NSLOT = CORES * BLOCKS * SLOTSB
    idx_flat = np.zeros(NSLOT, np.int16)          # pad gathers row 0
    dst_flat = np.full(NSLOT, -1.0, np.float32)   # pad never matches iota
    val_flat = np.zeros(NSLOT, np.float32)        # pad scales to 0
    idx_flat[slot] = lidx[order]
    dst_flat[slot] = dest[order]
    val_flat[slot] = ev[order]

    slots = idx_flat.reshape(CORES, NGROUPS, G, SLOTSB)
    dsts = dst_flat.reshape(CORES, NGROUPS, G, SLOTSB)
    vals = val_flat.reshape(CORES, NGROUPS, G, SLOTSB)

    # gather idx per call (g, q): [G*SLq] block-major; wrap to [128, ./16]
    gi_parts = []
    for q in range(Q):
        arr = slots[:, :, :, qslotoff[q] : qslotoff[q] + SLq[q]]
        arr = np.ascontiguousarray(arr).reshape(CORES, NGROUPS, G * int(SLq[q]))
        w16 = arr.reshape(CORES, NGROUPS, -1, 16)
        w16 = np.moveaxis(w16, 3, 2)             # [C, NGR, 16, CALLE/16]
        gi_parts.append(np.tile(w16, (1, 1, 8, 1)))
    gidx = np.ascontiguousarray(np.concatenate(gi_parts, axis=3))

    # dst/val in chunk-column layout [C, NGR, 128, G*NCH]:
    # column (lb, q, c) = lb*NCH + qchunkoff[q] + c
    def to_cols(a):
        parts = []
        for q in range(Q):
            seg = a[:, :, :, qslotoff[q] : qslotoff[q] + SLq[q]]
            parts.append(
                np.ascontiguousarray(seg).reshape(
                    CORES, NGROUPS, G, int(Cq[q]), 128
                )
            )
        cols = np.concatenate(parts, axis=3)      # [C, NGR, G, NCH, 128]
        cols = cols.reshape(CORES, NGROUPS, G * NCH, 128)
        return np.ascontiguousarray(np.moveaxis(cols, 3, 2))

    gdst = to_cols(dsts)
    gval = to_cols(vals)

    x_pad = np.zeros((int(QS[-1]), D), np.float32)
    x_pad[:N] = x
    irep = np.broadcast_to(np.arange(128, dtype=np.float32), (128, 128)).copy()
    ident = np.eye(128, dtype=np.float32)

    in_maps = []
    for k in range(CORES):
        in_maps.append(
            {
                "xq": x_pad,
                "w": np.ascontiguousarray(weight),
                "irep": irep,
                "ident": ident,
                "gidx": np.ascontiguousarray(gidx[k]),
                "gdst": gdst[k],
                "gval": gval[k],
            }
        )
    return in_maps, tuple(int(c) for c in Cq), perm


# ------------------------------------------------------------- bass program
def _build(Cq):
    import concourse.bacc as bacc
    import concourse.mybir as mybir
    import concourse.tile as tile

    f32 = mybir.dt.float32
    i16 = mybir.dt.int16
    SLq = [c * 128 for c in Cq]
    NCH = sum(Cq)
    qchunkoff = [0]
    for c in Cq[:-1]:
        qchunkoff.append(qchunkoff[-1] + c)
    CALLE = [G * sl for sl in SLq]
    off16 = [0]
    for c in CALLE:
        off16.append(off16[-1] + c // 16)
    TOT16 = off16[-1]

    nc = bacc.Bacc(
        "TRN2", target_bir_lowering=False, debug=False, num_devices=CORES
    )
    NX = int(QS[-1])
    x_d = nc.dram_tensor("xq", [NX, D], f32, kind="ExternalInput")
    w_d = nc.dram_tensor("w", [D, D], f32, kind="ExternalInput")
    irep_d = nc.dram_tensor("irep", [128, 128], f32, kind="ExternalInput")
    id_d = nc.dram_tensor("ident", [128, 128], f32, kind="ExternalInput")
    gidx_d = nc.dram_tensor("gidx", [NGROUPS, 128, TOT16], i16, kind="ExternalInput")
    gdst_d = nc.dram_tensor(
        "gdst", [NGROUPS, 128, G * NCH], f32, kind="ExternalInput"
    )
    gval_d = nc.dram_tensor(
        "gval", [NGROUPS, 128, G * NCH], f32, kind="ExternalInput"
    )
    outT_d = nc.dram_tensor("outT", [D, RPC], f32, kind="ExternalOutput")

    eq = mybir.AluOpType.is_equal
    mul = mybir.AluOpType.mult

    with tile.TileContext(nc) as tc:
        with (
            tc.tile_pool(name="const", bufs=1) as cpool,
            tc.tile_pool(name="io", bufs=3) as iopool,
            tc.tile_pool(name="vh", bufs=3) as vhpool,
            tc.tile_pool(name="sb", bufs=4) as sbpool,
            tc.tile_pool(name="outsb", bufs=1) as opool,
            tc.tile_pool(name="pa", bufs=3, space="PSUM") as papool,
            tc.tile_pool(name="pt", bufs=2, space="PSUM") as ptpool,
            tc.tile_pool(name="po", bufs=2, space="PSUM") as popool,
        ):
            w_sb = cpool.tile([D, D], f32, name="w_sb")
            irep_sb = cpool.tile([128, 128], f32, name="irep_sb")
            id_sb = cpool.tile([128, 128], f32, name="id_sb")
            outT_sb = opool.tile([D, RPC], f32, name="outT_sb")

            nc.sync.dma_start(out=w_sb[:], in_=w_d[:])
            nc.sync.dma_start(out=irep_sb[:], in_=irep_d[:])
            nc.sync.dma_start(out=id_sb[:], in_=id_d[:])

            # persistent double-buffered msgs tiles (gather fills every slot;
            # idx pads gather row 0, so contents are always finite)
            NB = 2
            msgs_t = [
                [
                    cpool.tile([128, G, Cq[q], D], f32, name=f"msgs{bi}_{q}")
                    for q in range(Q)
                ]
                for bi in range(NB)
            ]

            def emit_block(b, dst_t, rhs_fn):
                # one-hot for the whole block in one DVE op
                lb = b % G
                vh = vhpool.tile([128, NCH, 128], f32, tag="vh", name=f"vh{b}")
                nc.vector.tensor_tensor(
                    vh[:],
                    irep_sb[:].unsqueeze(1).broadcast_to([128, NCH, 128]),
                    dst_t[:, lb * NCH : (lb + 1) * NCH]
                    .unsqueeze(2)
                    .broadcast_to([128, NCH, 128]),
                    eq,
                )
                pa = papool.tile([128, D], f32, tag="pa", name=f"pa{b}")
                i = 0
                for q in range(Q):
                    for c in range(Cq[q]):
                        nc.tensor.matmul(
                            pa[:],
                            vh[:, qchunkoff[q] + c, :],
                            rhs_fn(q, c),
                            start=(i == 0),
                            stop=(i == NCH - 1),
                        )
                        i += 1
                agg_sb = sbpool.tile([128, D], f32, tag="agg", name=f"agg{b}")
                nc.vector.tensor_copy(agg_sb[:], pa[:])
                pt = ptpool.tile([D, 128], f32, tag="pt", name=f"pt{b}")
                nc.tensor.transpose(pt[:], agg_sb[:], id_sb[:])
                aggT_sb = sbpool.tile([D, 128], f32, tag="aggT", name=f"aggT{b}")
                nc.vector.tensor_copy(aggT_sb[:], pt[:])
                po = popool.tile([D, 128], f32, tag="po", name=f"po{b}")
                nc.tensor.matmul(po[:], w_sb[:], aggT_sb[:], start=True, stop=True)
                nc.vector.tensor_copy(outT_sb[:, b * 128 : (b + 1) * 128], po[:])

            for g in range(NGROUPS):
                idx_t = iopool.tile([128, TOT16], i16, tag="idx", name=f"idx{g}")
                dst_t = iopool.tile([128, G * NCH], f32, tag="dst", name=f"dst{g}")
                val_t = iopool.tile([128, G * NCH], f32, tag="val", name=f"val{g}")
                nc.sync.dma_start(out=idx_t[:], in_=gidx_d[g])
                nc.sync.dma_start(out=dst_t[:], in_=gdst_d[g])
                nc.sync.dma_start(out=val_t[:], in_=gval_d[g])

                if g < NGROUPS - 1:
                    msgs = msgs_t[g % NB]
                    for q in range(Q):
                        m = msgs[q]
                        nc.gpsimd.dma_gather(
                            m[:].rearrange("p g c d -> p (g c) d"),
                            x_d[int(QS[q]) : int(QS[q + 1]), :],
                            idx_t[:, off16[q] : off16[q + 1]],
                            CALLE[q],
                            CALLE[q],
                            D,
                            # single_packet=True needs the whole call inside
                            # the 1024-desc SWDGE ring -> crash on big calls
                            single_packet=False,
                        )
                        # scale msgs by edge_vals (broadcast along features);
                        # val=0 pads zero the padded slots
                        nc.vector.tensor_tensor(
                            m[:],
                            m[:],
                            val_t[:]
                            .rearrange("p (l n) -> p l n", l=G)[
                                :, :, qchunkoff[q] : qchunkoff[q] + Cq[q]
                            ]
                            .unsqueeze(3)
                            .broadcast_to([128, G, Cq[q], D]),
                            mul,
                        )
                    for lb in range(G):
                        b = g * G + lb
                        msgs_l = msgs
                        emit_block(
                            b, dst_t,
                            lambda q, c, _m=msgs_l, _lb=lb: _m[q][:, _lb, c, :],
                        )
                else:
                    # taper the final supergroup: the first G-2 blocks use
                    # grouped per-quarter calls (fewer 540ns call overheads);
                    # only the last 2 blocks use per-block calls so the
                    # kernel tail is one block rather than a whole supergroup
                    msgs = msgs_t[g % NB]
                    GH = G - 2
                    for q in range(Q):
                        nc.gpsimd.dma_gather(
                            msgs[q][:, :GH].rearrange("p g c d -> p (g c) d"),
                            x_d[int(QS[q]) : int(QS[q + 1]), :],
                            idx_t[:, off16[q] : off16[q] + GH * (SLq[q] // 16)],
                            GH * SLq[q],
                            GH * SLq[q],
                            D,
                            single_packet=False,
                        )
                        nc.vector.tensor_tensor(
                            msgs[q][:, :GH],
                            msgs[q][:, :GH],
                            val_t[:]
                            .rearrange("p (l n) -> p l n", l=G)[
                                :, :GH, qchunkoff[q] : qchunkoff[q] + Cq[q]
                            ]
                            .unsqueeze(3)
                            .broadcast_to([128, GH, Cq[q], D]),
                            mul,
                        )
                    for lb in range(GH):
                        b = g * G + lb
                        emit_block(
                            b, dst_t, lambda q, c, _m=msgs, _lb=lb: _m[q][:, _lb, c, :]
                        )
                    for lb in range(GH, G):
                        # dedicated tiles: no aliasing with the grouped msgs
                        # tile, so blocks 0..GH-1 emit while these gather
                        mt = [
                            sbpool.tile(
                                [128, Cq[q], D], f32,
                                tag=f"mt{lb - GH}_{q}", name=f"mt{lb}_{q}",
                            )
                            for q in range(Q)
                        ]
                        for q in range(Q):
                            nc.gpsimd.dma_gather(
                                mt[q][:],
                                x_d[int(QS[q]) : int(QS[q + 1]), :],
                                idx_t[
                                    :,
                                    off16[q]
                                    + lb * (SLq[q] // 16) : off16[q]
                                    + (lb + 1) * (SLq[q] // 16),
                                ],
                                SLq[q],
                                SLq[q],
                                D,
                                single_packet=False,
                            )
                            nc.vector.tensor_tensor(
                                mt[q][:],
                                mt[q][:],
                                val_t[
                                    :,
                                    lb * NCH
                                    + qchunkoff[q] : lb * NCH
                                    + qchunkoff[q]
                                    + Cq[q],
                                ]
                                .unsqueeze(2)
                                .broadcast_to([128, Cq[q], D]),
                                mul,
                            )
                        b = g * G + lb
                        emit_block(
                            b, dst_t, lambda q, c, _m=mt: _m[q][:, c, :]
                        )
                nc.sync.dma_start(
                    out=outT_d[:, g * G * 128 : (g + 1) * G * 128],
                    in_=outT_sb[:, g * G * 128 : (g + 1) * G * 128],
                )

    nc.compile()
    return nc


# ----------------------------------------------------------------- kernel()
def _ensure_ntff_hook():
    """Provide antenv.axon_hooks (absent in this image) so that
    run_bass_kernel_spmd's BASS_TRACE path can register the axon NTFF
    profiler instead of crashing on import."""
    try:
        import antenv.axon_hooks  # noqa: F401

        return
    except ImportError:
        pass
    import types

    import antenv

    mod = types.ModuleType("antenv.axon_hooks")
    holder = {"hook": None}
    mod.set_axon_ntff_profile_hook = lambda h: holder.__setitem__("hook", h)
    mod.get_axon_ntff_profile_hook = lambda: holder["hook"]
    sys.modules["antenv.axon_hooks"] = mod
    antenv.axon_hooks = mod
    try:
        from trn_agent_boot.trn_boot import _ntff_profile_via_ctypes

        mod.set_axon_ntff_profile_hook(
            _ntff_profile_via_ctypes("/opt/axon/libaxon_pjrt.so")
        )
    except Exception:
        pass


def kernel(x, weight, edge_vals, edge_row, edge_col):
    global LAST_EXEC_TIME_NS
    from concourse.bass_utils import run_bass_kernel_spmd

    if os.environ.get("BASS_TRACE"):
        _ensure_ntff_hook()

    in_maps, Cq, perm = _prep(x, weight, edge_vals, edge_row, edge_col)
    if Cq not in _CACHE:
        _CACHE[Cq] = _build(Cq)
    nc = _CACHE[Cq]

    res = run_bass_kernel_spmd(nc, in_maps, list(range(CORES)))
    LAST_EXEC_TIME_NS = res.exec_time_ns

    out = np.empty((CORES * RPC, D), np.float32)
    for k in range(CORES):
        out[perm[k]] = res.results[k]["outT"].T
    return np.ascontiguousarray(out[:N])



# revision 22
# speedup vs baseline: 1.0515x; 1.0515x over previous
"""GCNConv (COO SpMM aggregation + dense GEMM) on 8 Trainium2 NeuronCores.

  msgs = edge_vals[:, None] * x[edge_col]          # [E, 64] gather+scale
  agg  = segment_sum(msgs, edge_row, N)            # [N, 64] scatter-add
  out  = agg @ weight                              # [N, 64] GEMM

Sharding: destination-node sharding (each core owns a contiguous row slab and
all edges targeting it) -> zero collectives.

The throughput limit is SWDGE descriptor generation for the per-edge row
gather (~7.75 ns/descriptor + ~540 ns/call on the Q7s), so the host-side prep
minimizes padded gather slots:
  - x is split into 4 unequal quarters [23040,23040,23040,31232] (int16
    gather indices), sized so each (block, quarter) edge-group mean sits
    well below a multiple of 128.
  - each core's 12544 rows are bin-packed into 98 blocks of 128 rows,
    balancing all 4 per-quarter degree sums, so the max (block, quarter)
    group stays within [512,512,512,768] slots (vs 640 for equal quarters
    without packing).  The row permutation is undone on the host at the end.

Per core:
  - gpsimd.dma_gather pulls 64-float source rows (256B each) from HBM
  - one DVE tensor_tensor per (supergroup, quarter) scales msgs by edge_vals
  - one DVE tensor_tensor per dest block builds the one-hot
    oh[e, r] = (dest[e] == r) via iota-compare with a broadcast dest AP
  - TensorE per 128-edge chunk: psum[128 rows, 64] += oh.T @ msgs
  - per block: PE-transpose agg -> aggT, then outT[64,128] = W.T @ aggT
  - one contiguous [64, rows] output DMA; host scatters rows back.
"""

import os
import sys

import numpy as np

if "/opt/trn_rl_repo" not in sys.path:
    sys.path.insert(0, "/opt/trn_rl_repo")

# ---------------------------------------------------------------- constants
N = 100000
E = 1600000
D = 64
CORES = 8
RPC = 12544          # rows per core (8*12544 = 100352 >= N)
BLOCKS = RPC // 128  # 98 dest blocks per core
Q = 4
QS = np.array([0, 30134, 54243, 78352, 100352], dtype=np.int64)  # quarter bounds
CAPQ = np.array([640, 512, 512, 512], dtype=np.int64)  # packing targets
CAPB3 = (512, 512, 512, 512, 512, 384, 384)  # per-lb Q3 caps within each sg
G = 7                # dest blocks per gather super-group (98 = 14*7)
NGROUPS = BLOCKS // G

LAST_EXEC_TIME_NS = None
_CACHE = {}


def _pack_rows(deg, caps):
    """Assign nb*128 rows (deg: [nb*128, 4] per-quarter degrees) to nb blocks
    of 128, balancing all 4 quarter sums against per-quarter caps.  Greedy
    rounds (one row per block per round) + peak-shaving swap repair.
    Returns (perm_local[pos] = row, feasible)."""
    nb = deg.shape[0] // 128
    order = np.argsort(-deg.sum(1), kind="stable")
    cur = np.zeros((nb, Q), np.float64)
    capf = caps.astype(np.float64)
    blk_of = np.empty(nb * 128, np.int64)
    for rnd in range(128):
        batch = order[rnd * nb : (rnd + 1) * nb]
        bscore = (deg[batch] / capf).max(1)
        bo = batch[np.argsort(-bscore, kind="stable")]
        load = (cur / capf).max(1)
        blko = np.argsort(load, kind="stable")
        cur[blko] += deg[bo]
        blk_of[bo] = blko
    # repair: swap the heaviest row (in the hottest quarter) of the hottest
    # block with a light row of the coolest block
    loadi = np.zeros((nb, Q), np.int64)
    np.add.at(loadi, blk_of, deg)
    rows_in = [list(np.where(blk_of == b)[0]) for b in range(nb)]
    for _ in range(8000):
        nl = loadi / capf
        b, q = np.unravel_index(np.argmax(nl), nl.shape)
        b, q = int(b), int(q)
        if nl[b, q] <= 1.0:
            break
        cand = max(rows_in[b], key=lambda r: deg[r, q])
        tgt = int(np.argmin(nl[:, q] + (np.arange(nb) == b) * 10))
        cand2 = min(rows_in[tgt], key=lambda r: deg[r, q])
        loadi[b] += deg[cand2] - deg[cand]
        loadi[tgt] += deg[cand] - deg[cand2]
        rows_in[b].remove(cand)
        rows_in[b].append(cand2)
        rows_in[tgt].remove(cand2)
        rows_in[tgt].append(cand)
    feasible = bool((loadi <= caps[None, :]).all())
    perm_local = np.empty(nb * 128, np.int64)
    for b in range(nb):
        for j, r in enumerate(rows_in[b]):
            perm_local[b * 128 + j] = r
    return perm_local, feasible


# ---------------------------------------------------------------- host prep
def _prep(x, weight, edge_vals, edge_row, edge_col):
    e_row = np.asarray(edge_row, dtype=np.int64)
    e_col = np.asarray(edge_col, dtype=np.int64)
    ev = np.asarray(edge_vals, dtype=np.float32)
    x = np.asarray(x, dtype=np.float32)
    weight = np.asarray(weight, dtype=np.float32)
    ne = e_row.shape[0]
    NPAD = CORES * RPC

    qq = np.searchsorted(QS, e_col, side="right") - 1
    lidx = (e_col - QS[qq]).astype(np.int16)

    # per-row per-quarter degrees -> per-core packing permutation.
    # Q3 gets per-lb caps (CAPB3: 2 light 384-slot blocks per sg, shaving
    # 28 pad chunks/core).  Phased packing: pick a q3-degree window of rows
    # whose q3 sum fits the light blocks AND leaves the big blocks under
    # cap, then pack each pool with the uniform-cap greedy.  Fall back to
    # uniform 512 caps if either pool packing is infeasible.
    deg_flat = np.bincount(e_row * Q + qq, minlength=NPAD * Q).reshape(NPAD, Q)
    nsm = sum(1 for c in CAPB3 if c < 512)       # light blocks per sg
    smpos = [lb for lb in range(G) if CAPB3[lb] < 512]
    capb3 = CAPB3
    caps_sm = np.array([CAPQ[0], CAPQ[1], CAPQ[2], 384], np.int64)
    caps_bg = CAPQ
    perm = np.empty((CORES, RPC), np.int64)      # perm[k, pos] = global row
    pos_of_row = np.empty(NPAD, np.int64)        # core-local position
    ok = True
    for k in range(CORES):
        degk = deg_flat[k * RPC : (k + 1) * RPC]
        nrs = nsm * NGROUPS * 128                # rows in the light pool
        nbb = (G - nsm) * NGROUPS                # big blocks
        o3 = np.argsort(degk[:, 3], kind="stable")
        csum = np.concatenate([[0], np.cumsum(degk[o3, 3])])
        win = csum[nrs:] - csum[:-nrs]           # q3 sum of window at i
        q3tot = int(degk[:, 3].sum())
        hi = nsm * NGROUPS * 384 - nsm * NGROUPS * 14
        lo = q3tot - (nbb * 512 - nbb * 14)
        target = (max(lo, 0) + hi) / 2.0
        i0 = int(np.clip(np.searchsorted(win, target), 0, len(win) - 1))
        sm_rows = o3[i0 : i0 + nrs]
        bg_rows = np.concatenate([o3[:i0], o3[i0 + nrs :]])
        pl_s, fs = _pack_rows(degk[sm_rows], caps_sm)
        pl_b, fb = _pack_rows(degk[bg_rows], caps_bg)
        if not (fs and fb):
            ok = False
            break
        pl = np.empty(RPC, np.int64)
        bj = sj = 0
        for s_ in range(NGROUPS):
            for lb in range(G):
                dstpos = (s_ * G + lb) * 128
                if lb in smpos:
                    pl[dstpos : dstpos + 128] = sm_rows[
                        pl_s[sj * 128 : (sj + 1) * 128]
                    ]
                    sj += 1
                else:
                    pl[dstpos : dstpos + 128] = bg_rows[
                        pl_b[bj * 128 : (bj + 1) * 128]
                    ]
                    bj += 1
        perm[k] = k * RPC + pl
        pos_of_row[perm[k]] = np.arange(RPC)
    if not ok:
        capb3 = (512,) * G                       # uniform fallback
        for k in range(CORES):
            pl, _ = _pack_rows(
                deg_flat[k * RPC : (k + 1) * RPC], caps_bg
            )
            perm[k] = k * RPC + pl
            pos_of_row[perm[k]] = np.arange(RPC)

    core = e_row // RPC
    pos = pos_of_row[e_row]
    blk = pos // 128
    dest = (pos % 128).astype(np.float32)

    # group counts -> per-quarter chunk counts (global static)
    gkey = (core * BLOCKS + blk) * Q + qq
    counts = np.bincount(gkey, minlength=CORES * BLOCKS * Q)
    cmaxb = counts.reshape(CORES, BLOCKS, Q).max(axis=0)  # [BLOCKS, Q]
    caps_blk = np.tile(CAPQ, (BLOCKS, 1))
    caps_blk[:, 3] = np.tile(np.array(capb3, np.int64), NGROUPS)
    assert (cmaxb <= caps_blk).all(), "packing exceeded caps"
    Cq = np.maximum(1, -(-cmaxb.max(axis=0) // 128))      # [Q] chunks
    SLq = Cq * 128
    SLOTSB = int(SLq.sum())                      # slots per block
    NCH = int(Cq.sum())                          # chunk-columns per block
    qslotoff = np.concatenate([[0], np.cumsum(SLq)[:-1]])

    order = np.argsort(gkey, kind="stable")
    NGK = CORES * BLOCKS * Q
    starts = np.zeros(NGK, np.int64)
    starts[1:] = np.cumsum(counts)[:-1]
    gsort = gkey[order]
    rank = np.arange(ne, dtype=np.int64) - starts[gsort]
    cb = gsort // Q
    qs = gsort % Q
    slot = cb * SLOTSB + qslotoff[qs] + rank

    NSLOT = CORES * BLOCKS * SLOTSB
    idx_flat = np.zeros(NSLOT, np.int16)          # pad gathers row 0
    dst_flat = np.full(NSLOT, -1.0, np.float32)   # pad never matches iota
    val_flat = np.zeros(NSLOT, np.float32)        # pad scales to 0
    idx_flat[slot] = lidx[order]
    dst_flat[slot] = dest[order]
    val_flat[slot] = ev[order]

    slots = idx_flat.reshape(CORES, NGROUPS, G, SLOTSB)
    dsts = dst_flat.reshape(CORES, NGROUPS, G, SLOTSB)
    vals = val_flat.reshape(CORES, NGROUPS, G, SLOTSB)

    # per-lb Q3 slot widths (light blocks only KEEP their first capb3 slots;
    # the dropped tail chunks are all-pad by cap construction)
    slq_b = np.tile(SLq, (G, 1))                 # [G, Q]
    slq_b[:, 3] = np.array(capb3, np.int64)
    cq_b = slq_b // 128                          # [G, Q] chunks per (lb, q)

    # gather idx per call (g, q): lb-major stream; wrap to [128, ./16]
    gi_parts = []
    for q in range(Q):
        pieces = [
            slots[:, :, lb, qslotoff[q] : qslotoff[q] + int(slq_b[lb, q])]
            for lb in range(G)
        ]
        arr = np.ascontiguousarray(np.concatenate(pieces, axis=2))
        w16 = arr.reshape(CORES, NGROUPS, -1, 16)
        w16 = np.moveaxis(w16, 3, 2)             # [C, NGR, 16, CALLE/16]
        gi_parts.append(np.tile(w16, (1, 1, 8, 1)))
    gidx = np.ascontiguousarray(np.concatenate(gi_parts, axis=3))

    # dst/val in chunk-column layout [C, NGR, 128, sum_lb NCH_b]:
    # columns ordered (lb, q, c) with per-lb Q3 chunk counts
    def to_cols(a):
        lb_parts = []
        for lb in range(G):
            for q in range(Q):
                seg = a[:, :, lb, qslotoff[q] : qslotoff[q] + int(slq_b[lb, q])]
                lb_parts.append(
                    np.ascontiguousarray(seg).reshape(
                        CORES, NGROUPS, int(cq_b[lb, q]), 128
                    )
                )
        cols = np.concatenate(lb_parts, axis=2)   # [C, NGR, sum NCH_b, 128]
        return np.ascontiguousarray(np.moveaxis(cols, 3, 2))

    gdst = to_cols(dsts)
    gval = to_cols(vals)

    x_pad = np.zeros((int(QS[-1]), D), np.float32)
    x_pad[:N] = x
    irep = np.broadcast_to(np.arange(128, dtype=np.float32), (128, 128)).copy()
    ident = np.eye(128, dtype=np.float32)

    in_maps = []
    for k in range(CORES):
        in_maps.append(
            {
                "xq": x_pad,
                "w": np.ascontiguousarray(weight),
                "irep": irep,
                "ident": ident,
                "gidx": np.ascontiguousarray(gidx[k]),
                "gdst": gdst[k],
                "gval": gval[k],
            }
        )
    return in_maps, (tuple(int(c) for c in Cq), tuple(int(v) for v in capb3)), perm


# ------------------------------------------------------------- bass program
def _build(key):
    import concourse.bacc as bacc
    import concourse.mybir as mybir
    import concourse.tile as tile

    Cq, capb3 = key
    f32 = mybir.dt.float32
    i16 = mybir.dt.int16
    SLq = [c * 128 for c in Cq]
    # per-lb (block-within-sg) tables; only Q3 varies by lb
    slq_b = [[SLq[0], SLq[1], SLq[2], capb3[lb]] for lb in range(G)]
    cq_b = [[sl // 128 for sl in row] for row in slq_b]
    nch_b = [sum(row) for row in cq_b]
    doff = [0]
    for v in nch_b:
        doff.append(doff[-1] + v)
    TOTNCH = doff[-1]
    qoff_b = [[0] * Q for _ in range(G)]
    for lb in range(G):
        for q in range(1, Q):
            qoff_b[lb][q] = qoff_b[lb][q - 1] + cq_b[lb][q - 1]
    moff3 = [0]
    for lb in range(G):
        moff3.append(moff3[-1] + cq_b[lb][3])
    CT3 = moff3[-1]
    CALLE = [sum(slq_b[lb][q] for lb in range(G)) for q in range(Q)]
    off16 = [0]
    for c in CALLE:
        off16.append(off16[-1] + c // 16)
    TOT16 = off16[-1]
    lboff16 = [[0] * (G + 1) for _ in range(Q)]
    for q in range(Q):
        for lb in range(G):
            lboff16[q][lb + 1] = lboff16[q][lb] + slq_b[lb][q] // 16

    nc = bacc.Bacc(
        "TRN2", target_bir_lowering=False, debug=False, num_devices=CORES
    )
    NX = int(QS[-1])
    x_d = nc.dram_tensor("xq", [NX, D], f32, kind="ExternalInput")
    w_d = nc.dram_tensor("w", [D, D], f32, kind="ExternalInput")
    irep_d = nc.dram_tensor("irep", [128, 128], f32, kind="ExternalInput")
    id_d = nc.dram_tensor("ident", [128, 128], f32, kind="ExternalInput")
    gidx_d = nc.dram_tensor("gidx", [NGROUPS, 128, TOT16], i16, kind="ExternalInput")
    gdst_d = nc.dram_tensor(
        "gdst", [NGROUPS, 128, TOTNCH], f32, kind="ExternalInput"
    )
    gval_d = nc.dram_tensor(
        "gval", [NGROUPS, 128, TOTNCH], f32, kind="ExternalInput"
    )
    outT_d = nc.dram_tensor("outT", [D, RPC], f32, kind="ExternalOutput")

    eq = mybir.AluOpType.is_equal
    mul = mybir.AluOpType.mult

    with tile.TileContext(nc) as tc:
        with (
            tc.tile_pool(name="const", bufs=1) as cpool,
            tc.tile_pool(name="io", bufs=3) as iopool,
            tc.tile_pool(name="vh", bufs=3) as vhpool,
            tc.tile_pool(name="sb", bufs=4) as sbpool,
            tc.tile_pool(name="outsb", bufs=1) as opool,
            tc.tile_pool(name="pa", bufs=3, space="PSUM") as papool,
            tc.tile_pool(name="pt", bufs=2, space="PSUM") as ptpool,
            tc.tile_pool(name="po", bufs=2, space="PSUM") as popool,
        ):
            w_sb = cpool.tile([D, D], f32, name="w_sb")
            irep_sb = cpool.tile([128, 128], f32, name="irep_sb")
            id_sb = cpool.tile([128, 128], f32, name="id_sb")
            outT_sb = opool.tile([D, RPC], f32, name="outT_sb")

            nc.sync.dma_start(out=w_sb[:], in_=w_d[:])
            nc.sync.dma_start(out=irep_sb[:], in_=irep_d[:])
            nc.sync.dma_start(out=id_sb[:], in_=id_d[:])

            # persistent double-buffered msgs tiles (gather fills every slot;
            # idx pads gather row 0, so contents are always finite).
            # q<3 keep the uniform [G, Cq] layout; q=3 is flat (per-lb widths)
            NB = 2
            msgs_t = [
                [
                    cpool.tile([128, G, Cq[q], D], f32, name=f"msgs{bi}_{q}")
                    for q in range(3)
                ]
                + [cpool.tile([128, CT3, D], f32, name=f"msgs{bi}_3")]
                for bi in range(NB)
            ]

            NCHMAX = max(nch_b)

            def emit_block(b, dst_t, rhs_fn):
                # one-hot for the whole block in one DVE op
                lb = b % G
                nch = nch_b[lb]
                vh = vhpool.tile([128, NCHMAX, 128], f32, tag="vh", name=f"vh{b}")
                nc.vector.tensor_tensor(
                    vh[:, :nch],
                    irep_sb[:].unsqueeze(1).broadcast_to([128, nch, 128]),
                    dst_t[:, doff[lb] : doff[lb] + nch]
                    .unsqueeze(2)
                    .broadcast_to([128, nch, 128]),
                    eq,
                )
                pa = papool.tile([128, D], f32, tag="pa", name=f"pa{b}")
                i = 0
                for q in range(Q):
                    for c in range(cq_b[lb][q]):
                        nc.tensor.matmul(
                            pa[:],
                            vh[:, qoff_b[lb][q] + c, :],
                            rhs_fn(q, c),
                            start=(i == 0),
                            stop=(i == nch - 1),
                        )
                        i += 1
                agg_sb = sbpool.tile([128, D], f32, tag="agg", name=f"agg{b}")
                nc.vector.tensor_copy(agg_sb[:], pa[:])
                pt = ptpool.tile([D, 128], f32, tag="pt", name=f"pt{b}")
                nc.tensor.transpose(pt[:], agg_sb[:], id_sb[:])
                aggT_sb = sbpool.tile([D, 128], f32, tag="aggT", name=f"aggT{b}")
                nc.vector.tensor_copy(aggT_sb[:], pt[:])
                po = popool.tile([D, 128], f32, tag="po", name=f"po{b}")
                nc.tensor.matmul(po[:], w_sb[:], aggT_sb[:], start=True, stop=True)
                nc.vector.tensor_copy(outT_sb[:, b * 128 : (b + 1) * 128], po[:])

            for g in range(NGROUPS):
                idx_t = iopool.tile([128, TOT16], i16, tag="idx", name=f"idx{g}")
                dst_t = iopool.tile([128, TOTNCH], f32, tag="dst", name=f"dst{g}")
                val_t = iopool.tile([128, TOTNCH], f32, tag="val", name=f"val{g}")
                nc.sync.dma_start(out=idx_t[:], in_=gidx_d[g])
                nc.sync.dma_start(out=dst_t[:], in_=gdst_d[g])
                nc.sync.dma_start(out=val_t[:], in_=gval_d[g])

                if g < NGROUPS - 1:
                    msgs = msgs_t[g % NB]

                    def mview(q, lb, _m=msgs):
                        if q < 3:
                            return _m[q][:, lb, :, :]
                        return _m[3][:, moff3[lb] : moff3[lb + 1], :]

                    for q in range(Q):
                        out_ap = (
                            msgs[q][:].rearrange("p g c d -> p (g c) d")
                            if q < 3
                            else msgs[3][:]
                        )
                        nc.gpsimd.dma_gather(
                            out_ap,
                            x_d[int(QS[q]) : int(QS[q + 1]), :],
                            idx_t[:, off16[q] : off16[q + 1]],
                            CALLE[q],
                            CALLE[q],
                            D,
                            # single_packet=True needs the whole call inside
                            # the 1024-desc SWDGE ring -> crash on big calls
                            single_packet=False,
                        )
                        # scale msgs by edge_vals (broadcast along features);
                        # val=0 pads zero the padded slots
                        for lb in range(G):
                            cqs = cq_b[lb][q]
                            nc.vector.tensor_tensor(
                                mview(q, lb),
                                mview(q, lb),
                                val_t[
                                    :,
                                    doff[lb]
                                    + qoff_b[lb][q] : doff[lb]
                                    + qoff_b[lb][q]
                                    + cqs,
                                ]
                                .unsqueeze(2)
                                .broadcast_to([128, cqs, D]),
                                mul,
                            )
                    for lb in range(G):
                        b = g * G + lb
                        emit_block(
                            b, dst_t,
                            lambda q, c, _mv=mview, _lb=lb: _mv(q, _lb)[:, c, :],
                        )
                else:
                    # taper the final supergroup: per-block calls into
                    # dedicated ping-pong tiles so each block's compute
                    # overlaps the next block's gather, and the kernel tail
                    # is one block rather than a whole supergroup
                    msgs = msgs_t[g % NB]

                    def mview(q, lb, _m=msgs):
                        if q < 3:
                            return _m[q][:, lb, :, :]
                        return _m[3][:, moff3[lb] : moff3[lb + 1], :]

                    for lb in range(G):
                        for q in range(Q):
                            cqs = cq_b[lb][q]
                            nc.gpsimd.dma_gather(
                                mview(q, lb),
                                x_d[int(QS[q]) : int(QS[q + 1]), :],
                                idx_t[
                                    :,
                                    off16[q]
                                    + lboff16[q][lb] : off16[q]
                                    + lboff16[q][lb + 1],
                                ],
                                slq_b[lb][q],
                                slq_b[lb][q],
                                D,
                                single_packet=False,
                            )
                            nc.vector.tensor_tensor(
                                mview(q, lb),
                                mview(q, lb),
                                val_t[
                                    :,
                                    doff[lb]
                                    + qoff_b[lb][q] : doff[lb]
                                    + qoff_b[lb][q]
                                    + cqs,
                                ]
                                .unsqueeze(2)
                                .broadcast_to([128, cqs, D]),
                                mul,
                            )
                        b = g * G + lb
                        emit_block(
                            b, dst_t,
                            lambda q, c, _mv=mview, _lb=lb: _mv(q, _lb)[:, c, :],
                        )
                nc.sync.dma_start(
                    out=outT_d[:, g * G * 128 : (g + 1) * G * 128],
                    in_=outT_sb[:, g * G * 128 : (g + 1) * G * 128],
                )

    nc.compile()
    return nc


# ----------------------------------------------------------------- kernel()
def _ensure_ntff_hook():
    """Provide antenv.axon_hooks (absent in this image) so that
    run_bass_kernel_spmd's BASS_TRACE path can register the axon NTFF
    profiler instead of crashing on import."""
    try:
        import antenv.axon_hooks  # noqa: F401

        return
    except ImportError:
        pass
    import types

    import antenv

    mod = types.ModuleType("antenv.axon_hooks")
    holder = {"hook": None}
    mod.set_axon_ntff_profile_hook = lambda h: holder.__setitem__("hook", h)
    mod.get_axon_ntff_profile_hook = lambda: holder["hook"]
    sys.modules["antenv.axon_hooks"] = mod
    antenv.axon_hooks = mod
    try:
        from trn_agent_boot.trn_boot import _ntff_profile_via_ctypes

        mod.set_axon_ntff_profile_hook(
            _ntff_profile_via_ctypes("/opt/axon/libaxon_pjrt.so")
        )
    except Exception:
        pass


def kernel(x, weight, edge_vals, edge_row, edge_col):
    global LAST_EXEC_TIME_NS
    from concourse.bass_utils import run_bass_kernel_spmd

    if os.environ.get("BASS_TRACE"):
        _ensure_ntff_hook()

    in_maps, key, perm = _prep(x, weight, edge_vals, edge_row, edge_col)
    if key not in _CACHE:
        _CACHE[key] = _build(key)
    nc = _CACHE[key]

    res = run_bass_kernel_spmd(nc, in_maps, list(range(CORES)))
    LAST_EXEC_TIME_NS = res.exec_time_ns

    out = np.empty((CORES * RPC, D), np.float32)
    for k in range(CORES):
        out[perm[k]] = res.results[k]["outT"].T
    return np.ascontiguousarray(out[:N])



# revision 23
# speedup vs baseline: 1.1219x; 1.0670x over previous
"""GCNConv (COO SpMM aggregation + dense GEMM) on 8 Trainium2 NeuronCores.

  msgs = edge_vals[:, None] * x[edge_col]          # [E, 64] gather+scale
  agg  = segment_sum(msgs, edge_row, N)            # [N, 64] scatter-add
  out  = agg @ weight                              # [N, 64] GEMM

Sharding: destination-node sharding (each core owns a contiguous row slab and
all edges targeting it) -> zero collectives.

The throughput limit is SWDGE descriptor generation for the per-edge row
gather (~7.75 ns/descriptor + ~540 ns/call on the Q7s), so the host-side prep
minimizes padded gather slots:
  - x is split into 4 unequal quarters [23040,23040,23040,31232] (int16
    gather indices), sized so each (block, quarter) edge-group mean sits
    well below a multiple of 128.
  - each core's 12544 rows are bin-packed into 98 blocks of 128 rows,
    balancing all 4 per-quarter degree sums, so the max (block, quarter)
    group stays within [512,512,512,768] slots (vs 640 for equal quarters
    without packing).  The row permutation is undone on the host at the end.

Per core:
  - gpsimd.dma_gather pulls 64-float source rows (256B each) from HBM
  - one DVE tensor_tensor per (supergroup, quarter) scales msgs by edge_vals
  - one DVE tensor_tensor per dest block builds the one-hot
    oh[e, r] = (dest[e] == r) via iota-compare with a broadcast dest AP
  - TensorE per 128-edge chunk: psum[128 rows, 64] += oh.T @ msgs
  - per block: PE-transpose agg -> aggT, then outT[64,128] = W.T @ aggT
  - one contiguous [64, rows] output DMA; host scatters rows back.
"""

import os
import sys

import numpy as np

if "/opt/trn_rl_repo" not in sys.path:
    sys.path.insert(0, "/opt/trn_rl_repo")

# ---------------------------------------------------------------- constants
N = 100000
E = 1600000
D = 64
CORES = 8
RPC = 12544          # rows per core (8*12544 = 100352 >= N)
BLOCKS = RPC // 128  # 98 dest blocks per core
Q = 4
QS = np.array([0, 30134, 54243, 78352, 100352], dtype=np.int64)  # quarter bounds
CAPQ = np.array([640, 512, 512, 512], dtype=np.int64)  # packing targets
CAPB3 = (512, 512, 512, 512, 384, 384, 384)  # per-lb Q3 caps within each sg
G = 7                # dest blocks per gather super-group (98 = 14*7)
NGROUPS = BLOCKS // G

LAST_EXEC_TIME_NS = None
_CACHE = {}


def _pack_rows(deg, caps):
    """Assign nb*128 rows (deg: [nb*128, 4] per-quarter degrees) to nb blocks
    of 128, balancing all 4 quarter sums against per-quarter caps.  Greedy
    rounds (one row per block per round) + peak-shaving swap repair.
    Returns (perm_local[pos] = row, feasible)."""
    nb = deg.shape[0] // 128
    order = np.argsort(-deg.sum(1), kind="stable")
    cur = np.zeros((nb, Q), np.float64)
    capf = caps.astype(np.float64)
    blk_of = np.empty(nb * 128, np.int64)
    for rnd in range(128):
        batch = order[rnd * nb : (rnd + 1) * nb]
        bscore = (deg[batch] / capf).max(1)
        bo = batch[np.argsort(-bscore, kind="stable")]
        load = (cur / capf).max(1)
        blko = np.argsort(load, kind="stable")
        cur[blko] += deg[bo]
        blk_of[bo] = blko
    # repair: swap the heaviest row (in the hottest quarter) of the hottest
    # block with a light row of the coolest block
    loadi = np.zeros((nb, Q), np.int64)
    np.add.at(loadi, blk_of, deg)
    rows_in = [list(np.where(blk_of == b)[0]) for b in range(nb)]
    for _ in range(8000):
        nl = loadi / capf
        b, q = np.unravel_index(np.argmax(nl), nl.shape)
        b, q = int(b), int(q)
        if nl[b, q] <= 1.0:
            break
        cand = max(rows_in[b], key=lambda r: deg[r, q])
        tgt = int(np.argmin(nl[:, q] + (np.arange(nb) == b) * 10))
        cand2 = min(rows_in[tgt], key=lambda r: deg[r, q])
        loadi[b] += deg[cand2] - deg[cand]
        loadi[tgt] += deg[cand] - deg[cand2]
        rows_in[b].remove(cand)
        rows_in[b].append(cand2)
        rows_in[tgt].remove(cand2)
        rows_in[tgt].append(cand)
    feasible = bool((loadi <= caps[None, :]).all())
    perm_local = np.empty(nb * 128, np.int64)
    for b in range(nb):
        for j, r in enumerate(rows_in[b]):
            perm_local[b * 128 + j] = r
    return perm_local, feasible


# ---------------------------------------------------------------- host prep
def _prep(x, weight, edge_vals, edge_row, edge_col):
    e_row = np.asarray(edge_row, dtype=np.int64)
    e_col = np.asarray(edge_col, dtype=np.int64)
    ev = np.asarray(edge_vals, dtype=np.float32)
    x = np.asarray(x, dtype=np.float32)
    weight = np.asarray(weight, dtype=np.float32)
    ne = e_row.shape[0]
    NPAD = CORES * RPC

    qq = np.searchsorted(QS, e_col, side="right") - 1
    lidx = (e_col - QS[qq]).astype(np.int16)

    # per-row per-quarter degrees -> per-core packing permutation.
    # Q3 gets per-lb caps (CAPB3: 2 light 384-slot blocks per sg, shaving
    # 28 pad chunks/core).  Phased packing: pick a q3-degree window of rows
    # whose q3 sum fits the light blocks AND leaves the big blocks under
    # cap, then pack each pool with the uniform-cap greedy.  Fall back to
    # uniform 512 caps if either pool packing is infeasible.
    deg_flat = np.bincount(e_row * Q + qq, minlength=NPAD * Q).reshape(NPAD, Q)
    nsm = sum(1 for c in CAPB3 if c < 512)       # light blocks per sg
    smpos = [lb for lb in range(G) if CAPB3[lb] < 512]
    capb3 = CAPB3
    caps_sm = np.array([CAPQ[0], CAPQ[1], CAPQ[2], 384], np.int64)
    caps_bg = CAPQ
    perm = np.empty((CORES, RPC), np.int64)      # perm[k, pos] = global row
    pos_of_row = np.empty(NPAD, np.int64)        # core-local position
    ok = True
    for k in range(CORES):
        degk = deg_flat[k * RPC : (k + 1) * RPC]
        nrs = nsm * NGROUPS * 128                # rows in the light pool
        nbb = (G - nsm) * NGROUPS                # big blocks
        o3 = np.argsort(degk[:, 3], kind="stable")
        csum = np.concatenate([[0], np.cumsum(degk[o3, 3])])
        win = csum[nrs:] - csum[:-nrs]           # q3 sum of window at i
        q3tot = int(degk[:, 3].sum())
        hi = nsm * NGROUPS * 384 - nsm * NGROUPS * 6
        lo = q3tot - (nbb * 512 - nbb * 6)
        target = (max(lo, 0) + hi) / 2.0
        i0 = int(np.clip(np.searchsorted(win, target), 0, len(win) - 1))
        sm_rows = o3[i0 : i0 + nrs]
        bg_rows = np.concatenate([o3[:i0], o3[i0 + nrs :]])
        pl_s, fs = _pack_rows(degk[sm_rows], caps_sm)
        pl_b, fb = _pack_rows(degk[bg_rows], caps_bg)
        if not (fs and fb):
            ok = False
            break
        pl = np.empty(RPC, np.int64)
        bj = sj = 0
        for s_ in range(NGROUPS):
            for lb in range(G):
                dstpos = (s_ * G + lb) * 128
                if lb in smpos:
                    pl[dstpos : dstpos + 128] = sm_rows[
                        pl_s[sj * 128 : (sj + 1) * 128]
                    ]
                    sj += 1
                else:
                    pl[dstpos : dstpos + 128] = bg_rows[
                        pl_b[bj * 128 : (bj + 1) * 128]
                    ]
                    bj += 1
        perm[k] = k * RPC + pl
        pos_of_row[perm[k]] = np.arange(RPC)
    if not ok:
        capb3 = (512,) * G                       # uniform fallback
        for k in range(CORES):
            pl, _ = _pack_rows(
                deg_flat[k * RPC : (k + 1) * RPC], caps_bg
            )
            perm[k] = k * RPC + pl
            pos_of_row[perm[k]] = np.arange(RPC)

    core = e_row // RPC
    pos = pos_of_row[e_row]
    blk = pos // 128
    dest = (pos % 128).astype(np.float32)

    # group counts -> per-quarter chunk counts (global static)
    gkey = (core * BLOCKS + blk) * Q + qq
    counts = np.bincount(gkey, minlength=CORES * BLOCKS * Q)
    cmaxb = counts.reshape(CORES, BLOCKS, Q).max(axis=0)  # [BLOCKS, Q]
    caps_blk = np.tile(CAPQ, (BLOCKS, 1))
    caps_blk[:, 3] = np.tile(np.array(capb3, np.int64), NGROUPS)
    assert (cmaxb <= caps_blk).all(), "packing exceeded caps"
    Cq = np.maximum(1, -(-cmaxb.max(axis=0) // 128))      # [Q] chunks
    SLq = Cq * 128
    SLOTSB = int(SLq.sum())                      # slots per block
    NCH = int(Cq.sum())                          # chunk-columns per block
    qslotoff = np.concatenate([[0], np.cumsum(SLq)[:-1]])

    order = np.argsort(gkey, kind="stable")
    NGK = CORES * BLOCKS * Q
    starts = np.zeros(NGK, np.int64)
    starts[1:] = np.cumsum(counts)[:-1]
    gsort = gkey[order]
    rank = np.arange(ne, dtype=np.int64) - starts[gsort]
    cb = gsort // Q
    qs = gsort % Q
    slot = cb * SLOTSB + qslotoff[qs] + rank

    NSLOT = CORES * BLOCKS * SLOTSB
    idx_flat = np.zeros(NSLOT, np.int16)          # pad gathers row 0
    dst_flat = np.full(NSLOT, -1.0, np.float32)   # pad never matches iota
    val_flat = np.zeros(NSLOT, np.float32)        # pad scales to 0
    idx_flat[slot] = lidx[order]
    dst_flat[slot] = dest[order]
    val_flat[slot] = ev[order]

    slots = idx_flat.reshape(CORES, NGROUPS, G, SLOTSB)
    dsts = dst_flat.reshape(CORES, NGROUPS, G, SLOTSB)
    vals = val_flat.reshape(CORES, NGROUPS, G, SLOTSB)

    # per-lb Q3 slot widths (light blocks only KEEP their first capb3 slots;
    # the dropped tail chunks are all-pad by cap construction)
    slq_b = np.tile(SLq, (G, 1))                 # [G, Q]
    slq_b[:, 3] = np.array(capb3, np.int64)
    cq_b = slq_b // 128                          # [G, Q] chunks per (lb, q)

    # gather idx per call (g, q): lb-major stream; wrap to [128, ./16]
    gi_parts = []
    for q in range(Q):
        pieces = [
            slots[:, :, lb, qslotoff[q] : qslotoff[q] + int(slq_b[lb, q])]
            for lb in range(G)
        ]
        arr = np.ascontiguousarray(np.concatenate(pieces, axis=2))
        w16 = arr.reshape(CORES, NGROUPS, -1, 16)
        w16 = np.moveaxis(w16, 3, 2)             # [C, NGR, 16, CALLE/16]
        gi_parts.append(np.tile(w16, (1, 1, 8, 1)))
    gidx = np.ascontiguousarray(np.concatenate(gi_parts, axis=3))

    # dst/val in chunk-column layout [C, NGR, 128, sum_lb NCH_b]:
    # columns ordered (lb, q, c) with per-lb Q3 chunk counts
    def to_cols(a):
        lb_parts = []
        for lb in range(G):
            for q in range(Q):
                seg = a[:, :, lb, qslotoff[q] : qslotoff[q] + int(slq_b[lb, q])]
                lb_parts.append(
                    np.ascontiguousarray(seg).reshape(
                        CORES, NGROUPS, int(cq_b[lb, q]), 128
                    )
                )
        cols = np.concatenate(lb_parts, axis=2)   # [C, NGR, sum NCH_b, 128]
        return np.ascontiguousarray(np.moveaxis(cols, 3, 2))

    gdst = to_cols(dsts)
    gval = to_cols(vals)

    x_pad = np.zeros((int(QS[-1]), D), np.float32)
    x_pad[:N] = x
    irep = np.broadcast_to(np.arange(128, dtype=np.float32), (128, 128)).copy()
    ident = np.eye(128, dtype=np.float32)

    in_maps = []
    for k in range(CORES):
        in_maps.append(
            {
                "xq": x_pad,
                "w": np.ascontiguousarray(weight),
                "irep": irep,
                "ident": ident,
                "gidx": np.ascontiguousarray(gidx[k]),
                "gdst": gdst[k],
                "gval": gval[k],
            }
        )
    return in_maps, (tuple(int(c) for c in Cq), tuple(int(v) for v in capb3)), perm


# ------------------------------------------------------------- bass program
def _build(key):
    import concourse.bacc as bacc
    import concourse.mybir as mybir
    import concourse.tile as tile

    Cq, capb3 = key
    f32 = mybir.dt.float32
    i16 = mybir.dt.int16
    SLq = [c * 128 for c in Cq]
    # per-lb (block-within-sg) tables; only Q3 varies by lb
    slq_b = [[SLq[0], SLq[1], SLq[2], capb3[lb]] for lb in range(G)]
    cq_b = [[sl // 128 for sl in row] for row in slq_b]
    nch_b = [sum(row) for row in cq_b]
    doff = [0]
    for v in nch_b:
        doff.append(doff[-1] + v)
    TOTNCH = doff[-1]
    qoff_b = [[0] * Q for _ in range(G)]
    for lb in range(G):
        for q in range(1, Q):
            qoff_b[lb][q] = qoff_b[lb][q - 1] + cq_b[lb][q - 1]
    moff3 = [0]
    for lb in range(G):
        moff3.append(moff3[-1] + cq_b[lb][3])
    CT3 = moff3[-1]
    CALLE = [sum(slq_b[lb][q] for lb in range(G)) for q in range(Q)]
    off16 = [0]
    for c in CALLE:
        off16.append(off16[-1] + c // 16)
    TOT16 = off16[-1]
    lboff16 = [[0] * (G + 1) for _ in range(Q)]
    for q in range(Q):
        for lb in range(G):
            lboff16[q][lb + 1] = lboff16[q][lb] + slq_b[lb][q] // 16

    nc = bacc.Bacc(
        "TRN2", target_bir_lowering=False, debug=False, num_devices=CORES
    )
    NX = int(QS[-1])
    x_d = nc.dram_tensor("xq", [NX, D], f32, kind="ExternalInput")
    w_d = nc.dram_tensor("w", [D, D], f32, kind="ExternalInput")
    irep_d = nc.dram_tensor("irep", [128, 128], f32, kind="ExternalInput")
    id_d = nc.dram_tensor("ident", [128, 128], f32, kind="ExternalInput")
    gidx_d = nc.dram_tensor("gidx", [NGROUPS, 128, TOT16], i16, kind="ExternalInput")
    gdst_d = nc.dram_tensor(
        "gdst", [NGROUPS, 128, TOTNCH], f32, kind="ExternalInput"
    )
    gval_d = nc.dram_tensor(
        "gval", [NGROUPS, 128, TOTNCH], f32, kind="ExternalInput"
    )
    outT_d = nc.dram_tensor("outT", [D, RPC], f32, kind="ExternalOutput")

    eq = mybir.AluOpType.is_equal
    mul = mybir.AluOpType.mult

    with tile.TileContext(nc) as tc:
        with (
            tc.tile_pool(name="const", bufs=1) as cpool,
            tc.tile_pool(name="io", bufs=3) as iopool,
            tc.tile_pool(name="vh", bufs=3) as vhpool,
            tc.tile_pool(name="sb", bufs=4) as sbpool,
            tc.tile_pool(name="outsb", bufs=1) as opool,
            tc.tile_pool(name="pa", bufs=3, space="PSUM") as papool,
            tc.tile_pool(name="pt", bufs=2, space="PSUM") as ptpool,
            tc.tile_pool(name="po", bufs=2, space="PSUM") as popool,
        ):
            w_sb = cpool.tile([D, D], f32, name="w_sb")
            irep_sb = cpool.tile([128, 128], f32, name="irep_sb")
            id_sb = cpool.tile([128, 128], f32, name="id_sb")
            outT_sb = opool.tile([D, RPC], f32, name="outT_sb")

            nc.sync.dma_start(out=w_sb[:], in_=w_d[:])
            nc.sync.dma_start(out=irep_sb[:], in_=irep_d[:])
            nc.sync.dma_start(out=id_sb[:], in_=id_d[:])

            # persistent double-buffered msgs tiles (gather fills every slot;
            # idx pads gather row 0, so contents are always finite).
            # q<3 keep the uniform [G, Cq] layout; q=3 is flat (per-lb widths)
            NB = 2
            msgs_t = [
                [
                    cpool.tile([128, G, Cq[q], D], f32, name=f"msgs{bi}_{q}")
                    for q in range(3)
                ]
                + [cpool.tile([128, CT3, D], f32, name=f"msgs{bi}_3")]
                for bi in range(NB)
            ]

            NCHMAX = max(nch_b)

            def emit_block(b, dst_t, rhs_fn):
                # one-hot for the whole block in one DVE op
                lb = b % G
                nch = nch_b[lb]
                vh = vhpool.tile([128, NCHMAX, 128], f32, tag="vh", name=f"vh{b}")
                nc.vector.tensor_tensor(
                    vh[:, :nch],
                    irep_sb[:].unsqueeze(1).broadcast_to([128, nch, 128]),
                    dst_t[:, doff[lb] : doff[lb] + nch]
                    .unsqueeze(2)
                    .broadcast_to([128, nch, 128]),
                    eq,
                )
                pa = papool.tile([128, D], f32, tag="pa", name=f"pa{b}")
                i = 0
                for q in range(Q):
                    for c in range(cq_b[lb][q]):
                        nc.tensor.matmul(
                            pa[:],
                            vh[:, qoff_b[lb][q] + c, :],
                            rhs_fn(q, c),
                            start=(i == 0),
                            stop=(i == nch - 1),
                        )
                        i += 1
                agg_sb = sbpool.tile([128, D], f32, tag="agg", name=f"agg{b}")
                nc.vector.tensor_copy(agg_sb[:], pa[:])
                pt = ptpool.tile([D, 128], f32, tag="pt", name=f"pt{b}")
                nc.tensor.transpose(pt[:], agg_sb[:], id_sb[:])
                aggT_sb = sbpool.tile([D, 128], f32, tag="aggT", name=f"aggT{b}")
                nc.vector.tensor_copy(aggT_sb[:], pt[:])
                po = popool.tile([D, 128], f32, tag="po", name=f"po{b}")
                nc.tensor.matmul(po[:], w_sb[:], aggT_sb[:], start=True, stop=True)
                nc.vector.tensor_copy(outT_sb[:, b * 128 : (b + 1) * 128], po[:])

            for g in range(NGROUPS):
                idx_t = iopool.tile([128, TOT16], i16, tag="idx", name=f"idx{g}")
                dst_t = iopool.tile([128, TOTNCH], f32, tag="dst", name=f"dst{g}")
                val_t = iopool.tile([128, TOTNCH], f32, tag="val", name=f"val{g}")
                nc.sync.dma_start(out=idx_t[:], in_=gidx_d[g])
                nc.sync.dma_start(out=dst_t[:], in_=gdst_d[g])
                nc.sync.dma_start(out=val_t[:], in_=gval_d[g])

                if g < NGROUPS - 1:
                    msgs = msgs_t[g % NB]

                    def mview(q, lb, _m=msgs):
                        if q < 3:
                            return _m[q][:, lb, :, :]
                        return _m[3][:, moff3[lb] : moff3[lb + 1], :]

                    for q in range(Q):
                        out_ap = (
                            msgs[q][:].rearrange("p g c d -> p (g c) d")
                            if q < 3
                            else msgs[3][:]
                        )
                        nc.gpsimd.dma_gather(
                            out_ap,
                            x_d[int(QS[q]) : int(QS[q + 1]), :],
                            idx_t[:, off16[q] : off16[q + 1]],
                            CALLE[q],
                            CALLE[q],
                            D,
                            # single_packet=True needs the whole call inside
                            # the 1024-desc SWDGE ring -> crash on big calls
                            single_packet=False,
                        )
                        # scale msgs by edge_vals (broadcast along features);
                        # val=0 pads zero the padded slots
                        for lb in range(G):
                            cqs = cq_b[lb][q]
                            nc.vector.tensor_tensor(
                                mview(q, lb),
                                mview(q, lb),
                                val_t[
                                    :,
                                    doff[lb]
                                    + qoff_b[lb][q] : doff[lb]
                                    + qoff_b[lb][q]
                                    + cqs,
                                ]
                                .unsqueeze(2)
                                .broadcast_to([128, cqs, D]),
                                mul,
                            )
                    for lb in range(G):
                        b = g * G + lb
                        emit_block(
                            b, dst_t,
                            lambda q, c, _mv=mview, _lb=lb: _mv(q, _lb)[:, c, :],
                        )
                else:
                    # taper the final supergroup: per-block calls into
                    # dedicated ping-pong tiles so each block's compute
                    # overlaps the next block's gather, and the kernel tail
                    # is one block rather than a whole supergroup
                    msgs = msgs_t[g % NB]

                    def mview(q, lb, _m=msgs):
                        if q < 3:
                            return _m[q][:, lb, :, :]
                        return _m[3][:, moff3[lb] : moff3[lb + 1], :]

                    for lb in range(G):
                        for q in range(Q):
                            cqs = cq_b[lb][q]
                            nc.gpsimd.dma_gather(
                                mview(q, lb),
                                x_d[int(QS[q]) : int(QS[q + 1]), :],
                                idx_t[
                                    :,
                                    off16[q]
                                    + lboff16[q][lb] : off16[q]
                                    + lboff16[q][lb + 1],
                                ],
                                slq_b[lb][q],
                                slq_b[lb][q],
                                D,
                                single_packet=False,
                            )
                            nc.vector.tensor_tensor(
                                mview(q, lb),
                                mview(q, lb),
                                val_t[
                                    :,
                                    doff[lb]
                                    + qoff_b[lb][q] : doff[lb]
                                    + qoff_b[lb][q]
                                    + cqs,
                                ]
                                .unsqueeze(2)
                                .broadcast_to([128, cqs, D]),
                                mul,
                            )
                        b = g * G + lb
                        emit_block(
                            b, dst_t,
                            lambda q, c, _mv=mview, _lb=lb: _mv(q, _lb)[:, c, :],
                        )
                nc.sync.dma_start(
                    out=outT_d[:, g * G * 128 : (g + 1) * G * 128],
                    in_=outT_sb[:, g * G * 128 : (g + 1) * G * 128],
                )

    nc.compile()
    return nc


# ----------------------------------------------------------------- kernel()
def _ensure_ntff_hook():
    """Provide antenv.axon_hooks (absent in this image) so that
    run_bass_kernel_spmd's BASS_TRACE path can register the axon NTFF
    profiler instead of crashing on import."""
    try:
        import antenv.axon_hooks  # noqa: F401

        return
    except ImportError:
        pass
    import types

    import antenv

    mod = types.ModuleType("antenv.axon_hooks")
    holder = {"hook": None}
    mod.set_axon_ntff_profile_hook = lambda h: holder.__setitem__("hook", h)
    mod.get_axon_ntff_profile_hook = lambda: holder["hook"]
    sys.modules["antenv.axon_hooks"] = mod
    antenv.axon_hooks = mod
    try:
        from trn_agent_boot.trn_boot import _ntff_profile_via_ctypes

        mod.set_axon_ntff_profile_hook(
            _ntff_profile_via_ctypes("/opt/axon/libaxon_pjrt.so")
        )
    except Exception:
        pass


def kernel(x, weight, edge_vals, edge_row, edge_col):
    global LAST_EXEC_TIME_NS
    from concourse.bass_utils import run_bass_kernel_spmd

    if os.environ.get("BASS_TRACE"):
        _ensure_ntff_hook()

    in_maps, key, perm = _prep(x, weight, edge_vals, edge_row, edge_col)
    if key not in _CACHE:
        _CACHE[key] = _build(key)
    nc = _CACHE[key]

    res = run_bass_kernel_spmd(nc, in_maps, list(range(CORES)))
    LAST_EXEC_TIME_NS = res.exec_time_ns

    out = np.empty((CORES * RPC, D), np.float32)
    for k in range(CORES):
        out[perm[k]] = res.results[k]["outT"].T
    return np.ascontiguousarray(out[:N])

